# revision 11
# baseline (speedup 1.0000x reference)
"""Trainium2 Bass kernel for the DQN topk-masking problem.

Computes, for the full batch:
    h1 = relu(x @ W1 + b1); h2 = relu(h1 @ W2 + b2); q = h2 @ W3 + b3
    out[i, j] = q[i, j] if j in possible_moves[i] else -inf
(reference also maps q==0 at legal positions to -inf; for continuous random
inputs that event has probability ~0 and is not special-cased here.)

Sharding: data-parallel over the batch dim across 8 NeuronCores; the small
MLP weights are replicated. Each core computes its 1024-row slice end to end;
no collectives.

Per-core structure (matmul datapath fp16, PSUM accumulation and output fp32):
  mask stream (starts immediately, pacing the kernel on GPSIMD): per
    (row-block, 1024-col chunk) local_scatter writes exp(b3[move])
    (host-precomputed fp16, bucketed per chunk) into a zeroed tile; the
    scalar engine takes Ln of 2048-wide pairs -- exactly b3 at legal
    positions, exactly -inf on the zero background.
  MLP: x^T via DMA-transpose (fp16); MM1/MM2 on the PE; bias+relu fused on
    DVE (per-partition bias AP), giving h2^T for all 1024 rows.
  merge: per (row-block, 2048-col tile) MM3 into PSUM, one DVE add
    (q_psum + ln_tile) -> masked, biased output tile, DMA'd out.
"""
import sys

sys.path.insert(0, "/opt/trn_rl_repo")

import numpy as np

import concourse.bacc as bacc
import concourse.mybir as mybir
from concourse.tile import TileContext
from concourse.bass_utils import run_bass_kernel_spmd

P = 128          # SBUF partitions
B = 8192         # full batch
IN = 2048        # input features
H = 256          # hidden width
A = 8192         # action count (output width)
K = 512          # moves per row
NCORES = 8
BL = B // NCORES            # rows per core (1024)
NBLK = BL // P              # 128-row blocks (8)
CW = 1024                   # scatter chunk width (local_scatter num_elems)
NCC = A // CW               # scatter chunks (8)
QW = 2048                   # q/output tile width (2 scatter chunks)
NQC = A // QW               # q tiles per row-block (4)
IN_CH = IN // P             # 16
H_CH = H // P               # 2
KB = 160                    # bucketed moves per (row, chunk); fallback 512

f32 = mybir.dt.float32
fp16 = mybir.dt.float16
i16 = mybir.dt.int16

_BUILT = {}


def _build(kb):
    nc = bacc.Bacc()

    x_d = nc.dram_tensor("x", [BL, IN], fp16, kind="ExternalInput")
    m_d = nc.dram_tensor("m", [BL, NCC, kb], i16, kind="ExternalInput")
    eb3_d = nc.dram_tensor("eb3", [BL, NCC, kb], fp16, kind="ExternalInput")
    w1_d = nc.dram_tensor("w1", [IN, H], fp16, kind="ExternalInput")
    b1_d = nc.dram_tensor("b1", [H], f32, kind="ExternalInput")
    w2_d = nc.dram_tensor("w2", [H, H], fp16, kind="ExternalInput")
    b2_d = nc.dram_tensor("b2", [H], f32, kind="ExternalInput")
    w3_d = nc.dram_tensor("w3", [H, A], fp16, kind="ExternalInput")
    out_d = nc.dram_tensor("out", [BL, A], f32, kind="ExternalOutput")

    with TileContext(nc) as tc:
        with (
            tc.tile_pool(name="const", bufs=1) as cp,
            tc.tile_pool(name="mrows", bufs=3) as mp,
            tc.tile_pool(name="mask", bufs=3) as maskp,
            tc.tile_pool(name="lnp", bufs=12) as lnp,
            tc.tile_pool(name="outp", bufs=4) as outp,
            tc.tile_pool(name="psum", bufs=4, space="PSUM") as psp,
        ):
            # ---- mask stream inputs first: scatters depend only on these
            m_tiles = {}
            for bi in range(NBLK):
                m_bi = mp.tile([P, NCC, kb], i16, tag="mrows")
                nc.sync.dma_start(out=m_bi[:], in_=m_d[bi * P:(bi + 1) * P, :, :])
                eb3_bi = mp.tile([P, NCC, kb], fp16, tag="eb3rows")
                nc.sync.dma_start(out=eb3_bi[:], in_=eb3_d[bi * P:(bi + 1) * P, :, :])
                m_tiles[bi] = (m_bi, eb3_bi)
                if bi == 0:
                    # weights/x follow the first mask rows on the load queue
                    w1_sb = cp.tile([P, IN_CH, H], fp16, tag="w1")
                    nc.sync.dma_start(
                        out=w1_sb[:], in_=w1_d[:].rearrange("(c p) h -> p c h", p=P)
                    )
                    w2_sb = cp.tile([P, H_CH, H], fp16, tag="w2")
                    nc.sync.dma_start(
                        out=w2_sb[:], in_=w2_d[:].rearrange("(c p) h -> p c h", p=P)
                    )
                    b1_sb = cp.tile([P, H_CH], f32, tag="b1")
                    nc.sync.dma_start(
                        out=b1_sb[:], in_=b1_d[:].rearrange("(c p) -> p c", p=P)
                    )
                    b2_sb = cp.tile([P, H_CH], f32, tag="b2")
                    nc.sync.dma_start(
                        out=b2_sb[:], in_=b2_d[:].rearrange("(c p) -> p c", p=P)
                    )
                    xt_sb = cp.tile([P, IN_CH, BL], fp16, tag="xt")
                    for c in range(IN_CH):
                        nc.sync.dma_start(
                            out=xt_sb[:, c, :],
                            in_=x_d[:, c * P:(c + 1) * P],
                            transpose=True,
                        )
                    w3_sb = cp.tile([P, H_CH, A], fp16, tag="w3")
                    nc.sync.dma_start(
                        out=w3_sb[:], in_=w3_d[:].rearrange("(c p) n -> p c n", p=P)
                    )

            # ---- mask stream: scatter + Ln for every (bi, qc), emitted first
            ln_tiles = {}
            for bi in range(NBLK):
                m_bi, eb3_bi = m_tiles[bi]
                for qc in range(NQC):
                    mask_t = maskp.tile([P, 2, CW], fp16, tag="mask")
                    for h in range(2):
                        cc = qc * 2 + h
                        nc.gpsimd.local_scatter(
                            out_ap=mask_t[:, h, :],
                            data_ap=eb3_bi[:, cc, :],
                            idxs_ap=m_bi[:, cc, :],
                            channels=P,
                            num_elems=CW,
                            num_idxs=kb,
                        )
                    ln_t = lnp.tile([P, QW], fp16, tag="ln")
                    nc.scalar.activation(
                        out=ln_t[:],
                        in_=mask_t[:].rearrange("p a b -> p (a b)"),
                        func=mybir.ActivationFunctionType.Ln,
                    )
                    ln_tiles[(bi, qc)] = ln_t

            h1t = cp.tile([P, H_CH, BL], fp16, tag="h1t")
            h2t = cp.tile([P, H_CH, BL], fp16, tag="h2t")

            # ---- MLP phase 1: h1^T, h2^T for all rows (relu+bias on DVE)
            for hc in range(H_CH):
                ps1 = psp.tile([P, BL], f32, space="PSUM", tag="ps")
                for half in range(2):
                    hs = slice(half * 512, (half + 1) * 512)
                    for c in range(IN_CH):
                        nc.tensor.matmul(
                            out=ps1[:, hs],
                            lhsT=w1_sb[:, c, hc * P:(hc + 1) * P],
                            rhs=xt_sb[:, c, hs],
                            start=(c == 0),
                            stop=(c == IN_CH - 1),
                        )
                nc.vector.tensor_scalar(
                    out=h1t[:, hc, :],
                    in0=ps1[:],
                    scalar1=b1_sb[:, hc:hc + 1],
                    scalar2=0.0,
                    op0=mybir.AluOpType.add,
                    op1=mybir.AluOpType.max,
                )
            for hc2 in range(H_CH):
                ps2 = psp.tile([P, BL], f32, space="PSUM", tag="ps")
                for half in range(2):
                    hs = slice(half * 512, (half + 1) * 512)
                    for hc in range(H_CH):
                        nc.tensor.matmul(
                            out=ps2[:, hs],
                            lhsT=w2_sb[:, hc, hc2 * P:(hc2 + 1) * P],
                            rhs=h1t[:, hc, hs],
                            start=(hc == 0),
                            stop=(hc == H_CH - 1),
                        )
                for half in range(2):
                    hs = slice(half * 512, (half + 1) * 512)
                    nc.vector.tensor_scalar(
                        out=h2t[:, hc2, hs],
                        in0=ps2[:, hs],
                        scalar1=b2_sb[:, hc2:hc2 + 1],
                        scalar2=0.0,
                        op0=mybir.AluOpType.add,
                        op1=mybir.AluOpType.max,
                    )

            # ---- phase 2: q chunks (1024-wide) + merge + store
            for bi in range(NBLK):
                for cc in range(NCC):
                    qc, h = cc // 2, cc % 2
                    psq = psp.tile([P, CW], f32, space="PSUM", tag="ps")
                    for hc2 in range(H_CH):
                        for ns in range(CW // 512):
                            nsl = slice(ns * 512, (ns + 1) * 512)
                            w3sl = slice(cc * CW + ns * 512, cc * CW + (ns + 1) * 512)
                            nc.tensor.matmul(
                                out=psq[:, nsl],
                                lhsT=h2t[:, hc2, bi * P:(bi + 1) * P],
                                rhs=w3_sb[:, hc2, w3sl],
                                start=(hc2 == 0),
                                stop=(hc2 == H_CH - 1),
                            )
                    out_t = outp.tile([P, CW], f32, tag="out")
                    nc.vector.tensor_tensor(
                        out=out_t[:],
                        in0=psq[:],
                        in1=ln_tiles[(bi, qc)][:, h * CW:(h + 1) * CW],
                        op=mybir.AluOpType.add,
                    )
                    nc.scalar.dma_start(
                        out=out_d[bi * P:(bi + 1) * P, cc * CW:(cc + 1) * CW],
                        in_=out_t[:],
                    )

    nc.compile()
    return nc


def _get_nc(kb=KB):
    if kb not in _BUILT:
        _BUILT[kb] = _build(kb)
    return _BUILT[kb]


def _bucket_moves(moves: np.ndarray, b3: np.ndarray, kb: int):
    """[n, K] move ids -> ([n, NCC, kb] int16 chunk-local indices, -1 padded,
    [n, NCC, kb] fp16 exp(b3[move])). None if a bucket exceeds kb."""
    n = moves.shape[0]
    cc_of = (moves >> 10).astype(np.int64)          # [n, K] in [0, NCC)
    rel = (moves & (CW - 1)).astype(np.int16)       # [n, K] in [0, CW)
    order = np.argsort(cc_of, axis=1, kind="stable")
    scc = np.take_along_axis(cc_of, order, axis=1)
    srel = np.take_along_axis(rel, order, axis=1)
    smov = np.take_along_axis(moves, order, axis=1)
    counts = np.zeros((n, NCC), dtype=np.int64)
    for c in range(NCC):
        counts[:, c] = (cc_of == c).sum(axis=1)
    if counts.max() > kb:
        return None
    starts = np.cumsum(counts, axis=1) - counts
    pos = np.arange(K)[None, :] - np.take_along_axis(starts, scc, axis=1)
    rows = np.arange(n)[:, None]
    buck = np.full((n, NCC, kb), -1, dtype=np.int16)
    buck[rows, scc, pos] = srel
    eb3 = np.exp(b3.astype(np.float64)).astype(np.float16)
    ebuck = np.zeros((n, NCC, kb), dtype=np.float16)
    ebuck[rows, scc, pos] = eb3[smov]
    return buck, ebuck


def _shard_inputs(inputs):
    x = np.ascontiguousarray(np.asarray(inputs["x"], dtype=np.float16))
    moves = np.asarray(inputs["possible_moves"]).astype(np.int64)
    W1 = np.ascontiguousarray(np.asarray(inputs["W1"], dtype=np.float16))
    b1 = np.ascontiguousarray(np.asarray(inputs["b1"], dtype=np.float32))
    W2 = np.ascontiguousarray(np.asarray(inputs["W2"], dtype=np.float16))
    b2 = np.ascontiguousarray(np.asarray(inputs["b2"], dtype=np.float32))
    W3 = np.ascontiguousarray(np.asarray(inputs["W3"], dtype=np.float16))
    b3 = np.asarray(inputs["b3"], dtype=np.float32).reshape(A)

    kb = KB
    r = _bucket_moves(moves, b3, kb)
    if r is None:
        kb = K
        r = _bucket_moves(moves, b3, kb)
        assert r is not None
    buck, ebuck = r
    buck = np.ascontiguousarray(buck)
    ebuck = np.ascontiguousarray(ebuck)

    in_maps = []
    for c in range(NCORES):
        sl = slice(c * BL, (c + 1) * BL)
        in_maps.append(
            {
                "x": x[sl],
                "m": buck[sl],
                "eb3": ebuck[sl],
                "w1": W1,
                "b1": b1,
                "w2": W2,
                "b2": b2,
                "w3": W3,
            }
        )
    return in_maps, kb


def kernel(**inputs) -> np.ndarray:
    in_maps, kb = _shard_inputs(inputs)
    nc = _get_nc(kb)
    res = run_bass_kernel_spmd(nc, in_maps, core_ids=list(range(NCORES)))
    return np.concatenate([r["out"] for r in res.results], axis=0)
